# revision 10
# baseline (speedup 1.0000x reference)
"""Trainium2 Bass kernel for nn_Part_CAM_mask.

Math: the reference builds, per part e and batch b, a gathered submatrix
chain rollout att[11] @ ... @ att[0] and keeps only row 0 of the product,
which it then multiplies with relu(features[s:t]).  Row 0 of a matrix chain
is a vector-matrix chain: v <- v @ X per layer, with v masked to the part's
token set after every step (equivalent to the gather).  Part 0 is dropped by
the final [:, 1:], so only parts e=1..4 are computed.

Precision: inputs are cast to fp16 on the host (halves DMA bytes, PE runs
1 cycle/row instead of fp32's two half-speed passes).  A per-step rescale
lambda=1/16 is folded into the 0/1 masks to keep v in fp16 range across the
12-step chain; the final CAM eviction multiplies by 16^12 to undo it.
Measured accuracy vs the f32 reference: ~9e-4 scale-relative absmax.

Layout per batch element (full 201-token space, no gathers):
  V [201part, 4free]; X [201part(i), 201free(j)] natural row-major.
  v_new[j] = sum_i X[i,j] v[i]  == matmul(out[j,n], lhsT=X[i,j], rhs=V[i,n])
  K=i and M=j chunked [0:128), [128:201) (all weight APs contiguous).
  The chain seed (rows 1..4 of layer 11, masked+scaled) is precomputed on
  the host and DMA'd straight into V.  Mask multiply happens during the
  PSUM->SBUF eviction.

Sharding: data-parallel over batch, 8 per core, no cross-core communication.
"""

import numpy as np
from contextlib import ExitStack

L, B, N, D = 12, 64, 201, 768
NCORES = 8
BPC = B // NCORES
# (e, s, t) for parts 1..4 with N=201, C=5
PARTS = [(1, 5, 201), (2, 5, 103), (3, 54, 152), (4, 103, 201)]
KA = 128
KB = N - KA  # 73
H = D // 2  # 384, psum bank-sized CAM split
LAM = 1.0 / 16.0
SCALE_BACK = float(16.0 ** 12)

_cache = {}
# Extra kwargs for run_bass_kernel_spmd (used by the local test harness to
# enable NTFF tracing; empty by default).
RUN_KWARGS = {}


def _masks():
    cmask = np.zeros((N, 4), np.float32)
    fmask = np.zeros((N, 4), np.float32)
    for k, (e, s, t) in enumerate(PARTS):
        cmask[e, k] = 1.0
        cmask[s:t, k] = 1.0
        fmask[s:t, k] = 1.0
    return cmask * LAM, fmask * LAM


def _build_program():
    import concourse.tile as tile
    from concourse import bacc, mybir

    f32 = mybir.dt.float32
    f16 = mybir.dt.float16
    nc = bacc.Bacc()
    x_d = nc.declare_dram_parameter("x", [L - 1, BPC, N, N], f16, isOutput=False)
    f_d = nc.declare_dram_parameter("features", [BPC, N, D], f16, isOutput=False)
    v0_d = nc.declare_dram_parameter("v0", [BPC, N, 4], f16, isOutput=False)
    cm_d = nc.declare_dram_parameter("cmask", [N, 4], f32, isOutput=False)
    fm_d = nc.declare_dram_parameter("fmask", [N, 4], f32, isOutput=False)
    out_d = nc.declare_dram_parameter("out", [BPC, 4, D], f32, isOutput=True)

    LM = L - 1  # 11 chain layers

    with ExitStack() as ctx:
        tc = ctx.enter_context(tile.TileContext(nc))
        const = ctx.enter_context(tc.tile_pool(name="const", bufs=1))
        xpool = ctx.enter_context(tc.tile_pool(name="xpool", bufs=2))
        fpool = ctx.enter_context(tc.tile_pool(name="fpool", bufs=2))
        vpool = ctx.enter_context(tc.tile_pool(name="vpool", bufs=3))
        opool = ctx.enter_context(tc.tile_pool(name="opool", bufs=8))
        ppool = ctx.enter_context(tc.tile_pool(name="ppool", bufs=2, space="PSUM"))
        cpool = ctx.enter_context(tc.tile_pool(name="cpool", bufs=1, space="PSUM"))

        cmA = const.tile([KA, 4], f32)
        nc.gpsimd.dma_start(out=cmA, in_=cm_d[0:KA])
        cmB = const.tile([KB, 4], f32)
        nc.gpsimd.dma_start(out=cmB, in_=cm_d[KA:N])
        fmA = const.tile([KA, 4], f32)
        nc.gpsimd.dma_start(out=fmA, in_=fm_d[0:KA])
        fmB = const.tile([KB, 4], f32)
        nc.gpsimd.dma_start(out=fmB, in_=fm_d[KA:N])

        for b in range(BPC):
            XA = xpool.tile([KA, LM, N], f16, tag="XA")
            XB = xpool.tile([KB, LM, N], f16, tag="XB")
            big_eng = nc.sync if b % 2 == 0 else nc.scalar
            small_eng = nc.scalar if b % 2 == 0 else nc.sync
            big_eng.dma_start(
                out=XA, in_=x_d[:, b, 0:KA, :].transpose([1, 0, 2])
            )
            small_eng.dma_start(
                out=XB, in_=x_d[:, b, KA:N, :].transpose([1, 0, 2])
            )
            FA = fpool.tile([KA, D], f16, tag="FA")
            small_eng.dma_start(out=FA, in_=f_d[b, 0:KA, :])
            FB = fpool.tile([KB, D], f16, tag="FB")
            big_eng.dma_start(out=FB, in_=f_d[b, KA:N, :])
            FRA = fpool.tile([KA, D], f16, tag="FRA")
            nc.vector.tensor_scalar_max(FRA, FA, 0.0)
            FRB = fpool.tile([KB, D], f16, tag="FRB")
            nc.vector.tensor_scalar_max(FRB, FB, 0.0)

            # chain seed: host-precomputed masked+scaled rows 1..4 of layer 11
            vA = vpool.tile([KA, 4], f16, tag="vA")
            nc.gpsimd.dma_start(out=vA, in_=v0_d[b, 0:KA])
            vB = vpool.tile([KB, 4], f16, tag="vB")
            nc.gpsimd.dma_start(out=vB, in_=v0_d[b, KA:N])

            for li in range(LM - 1, -1, -1):  # layers 10 .. 0
                pA = ppool.tile([KA, 4], f32, tag="pA")
                pB = ppool.tile([KB, 4], f32, tag="pB")
                nc.tensor.matmul(pA, lhsT=XA[:, li, 0:KA], rhs=vA, start=True, stop=False)
                nc.tensor.matmul(pA, lhsT=XB[:, li, 0:KA], rhs=vB, start=False, stop=True)
                nc.tensor.matmul(pB, lhsT=XA[:, li, KA:N], rhs=vA, start=True, stop=False)
                nc.tensor.matmul(pB, lhsT=XB[:, li, KA:N], rhs=vB, start=False, stop=True)
                mA, mB = (cmA, cmB) if li > 0 else (fmA, fmB)
                vA = vpool.tile([KA, 4], f16, tag="vA")
                vB = vpool.tile([KB, 4], f16, tag="vB")
                nc.vector.tensor_mul(vA, pA, mA)
                nc.vector.tensor_mul(vB, pB, mB)

            # CAM: out[4, 768] = V^T @ relu(F); rescale by 16^12 on eviction
            pc0 = cpool.tile([4, H], f32, tag="pc0")
            pc1 = cpool.tile([4, H], f32, tag="pc1")
            nc.tensor.matmul(pc0, lhsT=vA, rhs=FRA[:, 0:H], start=True, stop=False)
            nc.tensor.matmul(pc0, lhsT=vB, rhs=FRB[:, 0:H], start=False, stop=True)
            nc.tensor.matmul(pc1, lhsT=vA, rhs=FRA[:, H:D], start=True, stop=False)
            nc.tensor.matmul(pc1, lhsT=vB, rhs=FRB[:, H:D], start=False, stop=True)
            ob = opool.tile([4, D], f32, tag="ob")
            nc.vector.tensor_scalar_mul(ob[:, 0:H], pc0, SCALE_BACK)
            nc.vector.tensor_scalar_mul(ob[:, H:D], pc1, SCALE_BACK)
            nc.gpsimd.dma_start(out=out_d[b], in_=ob)

    nc.compile()
    return nc


def _get_program():
    if "nc" not in _cache:
        _cache["nc"] = _build_program()
    return _cache["nc"]


def kernel(**inputs):
    from concourse.bass_utils import run_bass_kernel_spmd

    x = np.asarray(inputs["x"])
    features = np.asarray(inputs["features"]).astype(np.float16)
    cmask, fmask = _masks()
    x16 = x.astype(np.float16)
    # host-side chain seed: rows 1..4 of layer 11, transposed to [b, N, 4],
    # masked and pre-scaled by lambda (lambda is a power of two, so doing
    # this in f32 then casting matches the on-chip order exactly)
    v0 = (x[L - 1, :, 1:5, :].transpose(0, 2, 1) * cmask[None]).astype(np.float16)
    nc = _get_program()
    in_maps = []
    for c in range(NCORES):
        sl = slice(c * BPC, (c + 1) * BPC)
        in_maps.append(
            {
                "x": np.ascontiguousarray(x16[: L - 1, sl]),
                "features": np.ascontiguousarray(features[sl]),
                "v0": np.ascontiguousarray(v0[sl]),
                "cmask": cmask,
                "fmask": fmask,
            }
        )
    res = run_bass_kernel_spmd(nc, in_maps, list(range(NCORES)), **RUN_KWARGS)
    _cache["last_result"] = res
    out = np.concatenate([res.results[c]["out"] for c in range(NCORES)], axis=0)
    return out.astype(np.float32)


# revision 11
# speedup vs baseline: 1.7526x; 1.7526x over previous
"""Trainium2 Bass kernel for nn_Part_CAM_mask.

Math: the reference builds, per part e and batch b, a gathered submatrix
chain rollout att[11] @ ... @ att[0] and keeps only row 0 of the product,
which it then multiplies with relu(features[s:t]).  Row 0 of a matrix chain
is a vector-matrix chain: v <- v @ X per layer, with v masked to the part's
token set after every step (equivalent to the gather).  Part 0 is dropped by
the final [:, 1:], so only parts e=1..4 are computed.

Precision: inputs are cast to fp16 on the host (halves DMA bytes, PE runs
1 cycle/row instead of fp32's two half-speed passes).  A per-step rescale
lambda=1/16 is folded into the 0/1 masks to keep v in fp16 range across the
12-step chain; the final CAM eviction multiplies by 16^12 to undo it.
Measured accuracy vs the f32 reference: ~8e-4 scale-relative absmax.

Layout per batch element (full 201-token space, no gathers):
  V [201part, 4free]; X [201part(i), 201free(j)].
  v_new[j] = sum_i X[i,j] v[i]  == matmul(out[j,n], lhsT=X[i,j], rhs=V[i,n])
  Token rows are stored pairwise (rows 2p,2p+1 on partition p) so the X DMA
  moves 804-byte contiguous runs; K and M are chunked even/odd (101+100),
  which keeps psum partition order consistent with the next layer's K order.
  The chain seed (rows 1..4 of layer 11, masked+scaled+permuted) comes
  precomputed from the host.  Mask multiply (masks pre-permuted even/odd)
  happens during the PSUM->SBUF eviction.

Sharding: data-parallel over batch, 8 per core, no cross-core communication.
"""

import numpy as np
from contextlib import ExitStack

L, B, N, D = 12, 64, 201, 768
NCORES = 8
BPC = B // NCORES
# (e, s, t) for parts 1..4 with N=201, C=5
PARTS = [(1, 5, 201), (2, 5, 103), (3, 54, 152), (4, 103, 201)]
KE = 101  # even rows 0,2,...,200
KO = 100  # odd rows 1,3,...,199
H = D // 2  # 384, psum bank-sized CAM split
LAM = 1.0 / 16.0
SCALE_BACK = float(16.0 ** 12)

_cache = {}
# Extra kwargs for run_bass_kernel_spmd (used by the local test harness to
# enable NTFF tracing; empty by default).
RUN_KWARGS = {}

PERM = np.concatenate([np.arange(0, N, 2), np.arange(1, N, 2)])


def _masks():
    cmask = np.zeros((N, 4), np.float32)
    fmask = np.zeros((N, 4), np.float32)
    for k, (e, s, t) in enumerate(PARTS):
        cmask[e, k] = 1.0
        cmask[s:t, k] = 1.0
        fmask[s:t, k] = 1.0
    cmask *= LAM
    fmask *= LAM
    return cmask, fmask


def _build_program():
    import concourse.tile as tile
    from concourse import bacc, mybir

    f32 = mybir.dt.float32
    f16 = mybir.dt.float16
    nc = bacc.Bacc()
    x_d = nc.declare_dram_parameter("x", [L - 1, BPC, N, N], f16, isOutput=False)
    f_d = nc.declare_dram_parameter("features", [BPC, N, D], f16, isOutput=False)
    v0_d = nc.declare_dram_parameter("v0", [BPC, N, 4], f16, isOutput=False)
    cm_d = nc.declare_dram_parameter("cmask", [N, 4], f32, isOutput=False)
    fm_d = nc.declare_dram_parameter("fmask", [N, 4], f32, isOutput=False)
    out_d = nc.declare_dram_parameter("out", [BPC, 4, D], f32, isOutput=True)

    LM = L - 1  # 11 chain layers

    with ExitStack() as ctx:
        tc = ctx.enter_context(tile.TileContext(nc))
        const = ctx.enter_context(tc.tile_pool(name="const", bufs=1))
        xpool = ctx.enter_context(tc.tile_pool(name="xpool", bufs=2))
        fpool = ctx.enter_context(tc.tile_pool(name="fpool", bufs=2))
        vpool = ctx.enter_context(tc.tile_pool(name="vpool", bufs=3))
        opool = ctx.enter_context(tc.tile_pool(name="opool", bufs=8))
        ppool = ctx.enter_context(tc.tile_pool(name="ppool", bufs=2, space="PSUM"))
        cpool = ctx.enter_context(tc.tile_pool(name="cpool", bufs=1, space="PSUM"))

        cmE = const.tile([KE, 4], f32)
        nc.gpsimd.dma_start(out=cmE, in_=cm_d[0:KE])
        cmO = const.tile([KO, 4], f32)
        nc.gpsimd.dma_start(out=cmO, in_=cm_d[KE:N])
        fmE = const.tile([KE, 4], f32)
        nc.gpsimd.dma_start(out=fmE, in_=fm_d[0:KE])
        fmO = const.tile([KO, 4], f32)
        nc.gpsimd.dma_start(out=fmO, in_=fm_d[KE:N])

        for b in range(BPC):
            # X chain layers, rows paired: partition p holds rows (2p, 2p+1)
            # as 402 contiguous fp16 elements per layer.
            XP = xpool.tile([KE, LM, 2 * N], f16, tag="XP")
            big_eng = nc.sync if b % 2 == 0 else nc.scalar
            small_eng = nc.scalar if b % 2 == 0 else nc.sync
            big_eng.dma_start(
                out=XP[0:KO, :, :],
                in_=x_d[:, b, 0 : N - 1, :].rearrange(
                    "l (p two) j -> p l (two j)", two=2
                ),
            )
            nc.gpsimd.dma_start(
                out=XP[KO:KE, :, 0:N],
                in_=x_d[:, b, N - 1 : N, :].transpose([1, 0, 2]),
            )
            # features, rows paired the same way
            FP = fpool.tile([KE, 2 * D], f16, tag="FP")
            small_eng.dma_start(
                out=FP[0:KO, :],
                in_=f_d[b, 0 : N - 1, :].rearrange("(p two) d -> p (two d)", two=2),
            )
            nc.gpsimd.dma_start(out=FP[KO:KE, 0:D], in_=f_d[b, N - 1 : N, :])
            FR = fpool.tile([KE, 2 * D], f16, tag="FR")
            nc.vector.tensor_scalar_max(FR, FP, 0.0)

            # chain seed: host-precomputed masked+scaled rows 1..4 of layer 11
            vE = vpool.tile([KE, 4], f16, tag="vE")
            nc.gpsimd.dma_start(out=vE, in_=v0_d[b, 0:KE])
            vO = vpool.tile([KO, 4], f16, tag="vO")
            nc.gpsimd.dma_start(out=vO, in_=v0_d[b, KE:N])

            for li in range(LM - 1, -1, -1):  # layers 10 .. 0
                pE = ppool.tile([KE, 4], f32, tag="pE")
                pO = ppool.tile([KO, 4], f32, tag="pO")
                # even output cols <- even-row X block + odd-row X block
                nc.tensor.matmul(pE, lhsT=XP[:, li, 0:N:2], rhs=vE, start=True, stop=False)
                nc.tensor.matmul(pE, lhsT=XP[0:KO, li, N : 2 * N : 2], rhs=vO, start=False, stop=True)
                # odd output cols
                nc.tensor.matmul(pO, lhsT=XP[:, li, 1:N:2], rhs=vE, start=True, stop=False)
                nc.tensor.matmul(pO, lhsT=XP[0:KO, li, N + 1 : 2 * N : 2], rhs=vO, start=False, stop=True)
                mE, mO = (cmE, cmO) if li > 0 else (fmE, fmO)
                vE = vpool.tile([KE, 4], f16, tag="vE")
                vO = vpool.tile([KO, 4], f16, tag="vO")
                nc.vector.tensor_mul(vE, pE, mE)
                nc.vector.tensor_mul(vO, pO, mO)

            # CAM: out[4, 768] = V^T @ relu(F); rescale by 16^12 on eviction
            pc0 = cpool.tile([4, H], f32, tag="pc0")
            pc1 = cpool.tile([4, H], f32, tag="pc1")
            nc.tensor.matmul(pc0, lhsT=vE, rhs=FR[:, 0:H], start=True, stop=False)
            nc.tensor.matmul(pc0, lhsT=vO, rhs=FR[0:KO, D : D + H], start=False, stop=True)
            nc.tensor.matmul(pc1, lhsT=vE, rhs=FR[:, H:D], start=True, stop=False)
            nc.tensor.matmul(pc1, lhsT=vO, rhs=FR[0:KO, D + H : 2 * D], start=False, stop=True)
            ob = opool.tile([4, D], f32, tag="ob")
            nc.vector.tensor_scalar_mul(ob[:, 0:H], pc0, SCALE_BACK)
            nc.vector.tensor_scalar_mul(ob[:, H:D], pc1, SCALE_BACK)
            nc.gpsimd.dma_start(out=out_d[b], in_=ob)

    nc.compile()
    return nc


def _get_program():
    if "nc" not in _cache:
        _cache["nc"] = _build_program()
    return _cache["nc"]


def kernel(**inputs):
    from concourse.bass_utils import run_bass_kernel_spmd

    x = np.asarray(inputs["x"])
    features = np.asarray(inputs["features"]).astype(np.float16)
    cmask, fmask = _masks()
    x16 = x[: L - 1].astype(np.float16)
    # host-side chain seed: rows 1..4 of layer 11 -> [b, N, 4], masked,
    # pre-scaled by lambda (a power of two, so order matches on-chip exactly),
    # and permuted to the even/odd on-chip row order.
    v0 = (x[L - 1, :, 1:5, :].transpose(0, 2, 1) * cmask[None]).astype(np.float16)
    v0 = np.ascontiguousarray(v0[:, PERM])
    cmaskp = np.ascontiguousarray(cmask[PERM])
    fmaskp = np.ascontiguousarray(fmask[PERM])
    nc = _get_program()
    in_maps = []
    for c in range(NCORES):
        sl = slice(c * BPC, (c + 1) * BPC)
        in_maps.append(
            {
                "x": np.ascontiguousarray(x16[:, sl]),
                "features": np.ascontiguousarray(features[sl]),
                "v0": np.ascontiguousarray(v0[sl]),
                "cmask": cmaskp,
                "fmask": fmaskp,
            }
        )
    res = run_bass_kernel_spmd(nc, in_maps, list(range(NCORES)), **RUN_KWARGS)
    _cache["last_result"] = res
    out = np.concatenate([res.results[c]["out"] for c in range(NCORES)], axis=0)
    return out.astype(np.float32)


# revision 12
# speedup vs baseline: 2.2398x; 1.2780x over previous
"""Trainium2 Bass kernel for nn_Part_CAM_mask.

Math: the reference builds, per part e and batch b, a gathered submatrix
chain rollout att[11] @ ... @ att[0] and keeps only row 0 of the product,
which it then multiplies with relu(features[s:t]).  Row 0 of a matrix chain
is a vector-matrix chain: v <- v @ X per layer, with v masked to the part's
token set after every step (equivalent to the gather).  Part 0 is dropped by
the final [:, 1:], so only parts e=1..4 are computed.

Precision: inputs are cast to fp16 on the host (halves DMA bytes, PE runs
1 cycle/row instead of fp32's two half-speed passes).  A per-step rescale
lambda=1/16 is folded into the 0/1 masks to keep v in fp16 range across the
12-step chain; the final CAM eviction multiplies by 16^12 to undo it.
Measured accuracy vs the f32 reference: ~8e-4 scale-relative absmax.

Layout per batch element (full 201-token space, no gathers):
  V [201part, 4free]; X [201part(i), 201free(j)].
  v_new[j] = sum_i X[i,j] v[i]  == matmul(out[j,n], lhsT=X[i,j], rhs=V[i,n])
  Token rows are stored pairwise (rows 2p,2p+1 on partition p) so the X DMA
  moves 804-byte contiguous runs; K and M are chunked even/odd (101+100),
  which keeps psum partition order consistent with the next layer's K order.
  The chain seed (rows 1..4 of layer 11, masked+scaled+permuted) comes
  precomputed from the host.  Mask multiply (masks pre-permuted even/odd)
  happens during the PSUM->SBUF eviction.

Sharding: data-parallel over batch, 8 per core, no cross-core communication.
"""

import numpy as np
from contextlib import ExitStack

L, B, N, D = 12, 64, 201, 768
NCORES = 8
BPC = B // NCORES
# (e, s, t) for parts 1..4 with N=201, C=5
PARTS = [(1, 5, 201), (2, 5, 103), (3, 54, 152), (4, 103, 201)]
KE = 101  # even rows 0,2,...,200
KO = 100  # odd rows 1,3,...,199
H = D // 2  # 384, psum bank-sized CAM split
LAM = 1.0 / 16.0
SCALE_BACK = float(16.0 ** 12)

_cache = {}
# Extra kwargs for run_bass_kernel_spmd (used by the local test harness to
# enable NTFF tracing; empty by default).
RUN_KWARGS = {}

PERM = np.concatenate([np.arange(0, N, 2), np.arange(1, N, 2)])


def _masks():
    cmask = np.zeros((N, 4), np.float32)
    fmask = np.zeros((N, 4), np.float32)
    for k, (e, s, t) in enumerate(PARTS):
        cmask[e, k] = 1.0
        cmask[s:t, k] = 1.0
        fmask[s:t, k] = 1.0
    cmask *= LAM
    fmask *= LAM
    return cmask, fmask


def _build_program():
    import concourse.tile as tile
    from concourse import bacc, mybir

    f32 = mybir.dt.float32
    f16 = mybir.dt.float16
    nc = bacc.Bacc()
    x_d = nc.declare_dram_parameter("x", [L - 1, BPC, N, N], f16, isOutput=False)
    f_d = nc.declare_dram_parameter("features", [BPC, N, D], f16, isOutput=False)
    v0_d = nc.declare_dram_parameter("v0", [BPC, N, 4], f16, isOutput=False)
    cm_d = nc.declare_dram_parameter("cmask", [N, 4], f32, isOutput=False)
    fm_d = nc.declare_dram_parameter("fmask", [N, 4], f32, isOutput=False)
    out_d = nc.declare_dram_parameter("out", [BPC, 4, D], f32, isOutput=True)

    LM = L - 1  # 11 chain layers

    with ExitStack() as ctx:
        tc = ctx.enter_context(tile.TileContext(nc))
        const = ctx.enter_context(tc.tile_pool(name="const", bufs=1))
        xpool = ctx.enter_context(tc.tile_pool(name="xpool", bufs=3))
        fpool = ctx.enter_context(tc.tile_pool(name="fpool", bufs=3))
        vpool = ctx.enter_context(tc.tile_pool(name="vpool", bufs=3))
        opool = ctx.enter_context(tc.tile_pool(name="opool", bufs=8))
        ppool = ctx.enter_context(tc.tile_pool(name="ppool", bufs=2, space="PSUM"))
        cpool = ctx.enter_context(tc.tile_pool(name="cpool", bufs=1, space="PSUM"))

        cmE = const.tile([KE, 4], f32)
        nc.sync.dma_start(out=cmE, in_=cm_d[0:KE])
        cmO = const.tile([KO, 4], f32)
        nc.sync.dma_start(out=cmO, in_=cm_d[KE:N])
        fmE = const.tile([KE, 4], f32)
        nc.sync.dma_start(out=fmE, in_=fm_d[0:KE])
        fmO = const.tile([KO, 4], f32)
        nc.sync.dma_start(out=fmO, in_=fm_d[KE:N])
        # all batches' chain seeds in one load, off the per-batch critical path
        V0E = const.tile([KE, BPC, 4], f16)
        nc.scalar.dma_start(out=V0E, in_=v0_d[:, 0:KE, :].transpose([1, 0, 2]))
        V0O = const.tile([KO, BPC, 4], f16)
        nc.scalar.dma_start(out=V0O, in_=v0_d[:, KE:N, :].transpose([1, 0, 2]))

        for b in range(BPC):
            # X chain layers, rows paired: partition p holds rows (2p, 2p+1)
            # as 402 contiguous fp16 elements per layer.
            XP = xpool.tile([KE, LM, 2 * N], f16, tag="XP")
            big_eng = nc.sync if b % 2 == 0 else nc.scalar
            small_eng = nc.scalar if b % 2 == 0 else nc.sync
            big_eng.dma_start(
                out=XP[0:KO, :, :],
                in_=x_d[:, b, 0 : N - 1, :].rearrange(
                    "l (p two) j -> p l (two j)", two=2
                ),
            )
            nc.gpsimd.dma_start(
                out=XP[KO:KE, :, 0:N],
                in_=x_d[:, b, N - 1 : N, :].transpose([1, 0, 2]),
            )
            # features, rows paired the same way
            FP = fpool.tile([KE, 2 * D], f16, tag="FP")
            small_eng.dma_start(
                out=FP[0:KO, :],
                in_=f_d[b, 0 : N - 1, :].rearrange("(p two) d -> p (two d)", two=2),
            )
            nc.gpsimd.dma_start(out=FP[KO:KE, 0:D], in_=f_d[b, N - 1 : N, :])
            FR = fpool.tile([KE, 2 * D], f16, tag="FR")
            nc.vector.tensor_scalar_max(FR, FP, 0.0)

            # chain seed: host-precomputed masked+scaled rows 1..4 of layer 11
            vE = V0E[:, b, :]
            vO = V0O[:, b, :]

            for li in range(LM - 1, -1, -1):  # layers 10 .. 0
                pE = ppool.tile([KE, 4], f32, tag="pE")
                pO = ppool.tile([KO, 4], f32, tag="pO")
                # even output cols <- even-row X block + odd-row X block
                nc.tensor.matmul(pE, lhsT=XP[:, li, 0:N:2], rhs=vE, start=True, stop=False)
                nc.tensor.matmul(pE, lhsT=XP[0:KO, li, N : 2 * N : 2], rhs=vO, start=False, stop=True)
                # odd output cols
                nc.tensor.matmul(pO, lhsT=XP[:, li, 1:N:2], rhs=vE, start=True, stop=False)
                nc.tensor.matmul(pO, lhsT=XP[0:KO, li, N + 1 : 2 * N : 2], rhs=vO, start=False, stop=True)
                mE, mO = (cmE, cmO) if li > 0 else (fmE, fmO)
                vE = vpool.tile([KE, 4], f16, tag="vE")
                vO = vpool.tile([KO, 4], f16, tag="vO")
                nc.vector.tensor_mul(vE, pE, mE)
                nc.vector.tensor_mul(vO, pO, mO)

            # CAM: out[4, 768] = V^T @ relu(F); rescale by 16^12 on eviction
            pc0 = cpool.tile([4, H], f32, tag="pc0")
            pc1 = cpool.tile([4, H], f32, tag="pc1")
            nc.tensor.matmul(pc0, lhsT=vE, rhs=FR[:, 0:H], start=True, stop=False)
            nc.tensor.matmul(pc0, lhsT=vO, rhs=FR[0:KO, D : D + H], start=False, stop=True)
            nc.tensor.matmul(pc1, lhsT=vE, rhs=FR[:, H:D], start=True, stop=False)
            nc.tensor.matmul(pc1, lhsT=vO, rhs=FR[0:KO, D + H : 2 * D], start=False, stop=True)
            ob = opool.tile([4, D], f32, tag="ob")
            nc.vector.tensor_scalar_mul(ob[:, 0:H], pc0, SCALE_BACK)
            nc.vector.tensor_scalar_mul(ob[:, H:D], pc1, SCALE_BACK)
            nc.gpsimd.dma_start(out=out_d[b], in_=ob)

    nc.compile()
    return nc


def _get_program():
    if "nc" not in _cache:
        _cache["nc"] = _build_program()
    return _cache["nc"]


def kernel(**inputs):
    from concourse.bass_utils import run_bass_kernel_spmd

    x = np.asarray(inputs["x"])
    features = np.asarray(inputs["features"]).astype(np.float16)
    cmask, fmask = _masks()
    x16 = x[: L - 1].astype(np.float16)
    # host-side chain seed: rows 1..4 of layer 11 -> [b, N, 4], masked,
    # pre-scaled by lambda (a power of two, so order matches on-chip exactly),
    # and permuted to the even/odd on-chip row order.
    v0 = (x[L - 1, :, 1:5, :].transpose(0, 2, 1) * cmask[None]).astype(np.float16)
    v0 = np.ascontiguousarray(v0[:, PERM])
    cmaskp = np.ascontiguousarray(cmask[PERM])
    fmaskp = np.ascontiguousarray(fmask[PERM])
    nc = _get_program()
    in_maps = []
    for c in range(NCORES):
        sl = slice(c * BPC, (c + 1) * BPC)
        in_maps.append(
            {
                "x": np.ascontiguousarray(x16[:, sl]),
                "features": np.ascontiguousarray(features[sl]),
                "v0": np.ascontiguousarray(v0[sl]),
                "cmask": cmaskp,
                "fmask": fmaskp,
            }
        )
    res = run_bass_kernel_spmd(nc, in_maps, list(range(NCORES)), **RUN_KWARGS)
    _cache["last_result"] = res
    out = np.concatenate([res.results[c]["out"] for c in range(NCORES)], axis=0)
    return out.astype(np.float32)


# revision 13
# speedup vs baseline: 2.5212x; 1.1256x over previous
"""Trainium2 Bass kernel for nn_Part_CAM_mask.

Math: the reference builds, per part e and batch b, a gathered submatrix
chain rollout att[11] @ ... @ att[0] and keeps only row 0 of the product,
which it then multiplies with relu(features[s:t]).  Row 0 of a matrix chain
is a vector-matrix chain: v <- v @ X per layer, with v masked to the part's
token set after every step (equivalent to the gather).  Part 0 is dropped by
the final [:, 1:], so only parts e=1..4 are computed.

Precision: inputs are cast to fp16 on the host (halves DMA bytes, PE runs
1 cycle/row instead of fp32's two half-speed passes).  A per-step rescale
lambda=1/16 is folded into the 0/1 masks to keep v in fp16 range across the
12-step chain; the final CAM eviction multiplies by 16^12 to undo it.
Measured accuracy vs the f32 reference: ~8e-4 scale-relative absmax.

Layout per batch element (full 201-token space, no gathers):
  V [201part, 4free]; X [201part(i), 201free(j)].
  v_new[j] = sum_i X[i,j] v[i]  == matmul(out[j,n], lhsT=X[i,j], rhs=V[i,n])
  Token rows are stored pairwise (rows 2p,2p+1 on partition p) so the X DMA
  moves 804-byte contiguous runs; K and M are chunked even/odd (101+100),
  which keeps psum partition order consistent with the next layer's K order.
  The chain seed (rows 1..4 of layer 11, masked+scaled+permuted) comes
  precomputed from the host.  Mask multiply (masks pre-permuted even/odd)
  happens during the PSUM->SBUF eviction.

Sharding: data-parallel over batch, 8 per core, no cross-core communication.
"""

import numpy as np
from contextlib import ExitStack

L, B, N, D = 12, 64, 201, 768
NCORES = 8
BPC = B // NCORES
# (e, s, t) for parts 1..4 with N=201, C=5
PARTS = [(1, 5, 201), (2, 5, 103), (3, 54, 152), (4, 103, 201)]
KE = 101  # even rows 0,2,...,200
KO = 100  # odd rows 1,3,...,199
H = D // 2  # 384, psum bank-sized CAM split
LAM = 1.0 / 16.0
SCALE_BACK = float(16.0 ** 12)

_cache = {}
# Extra kwargs for run_bass_kernel_spmd (used by the local test harness to
# enable NTFF tracing; empty by default).
RUN_KWARGS = {}

PERM = np.concatenate([np.arange(0, N, 2), np.arange(1, N, 2)])


def _masks():
    cmask = np.zeros((N, 4), np.float32)
    fmask = np.zeros((N, 4), np.float32)
    for k, (e, s, t) in enumerate(PARTS):
        cmask[e, k] = 1.0
        cmask[s:t, k] = 1.0
        fmask[s:t, k] = 1.0
    cmask *= LAM
    fmask *= LAM
    return cmask, fmask


def _build_program():
    import concourse.tile as tile
    from concourse import bacc, mybir

    f32 = mybir.dt.float32
    f16 = mybir.dt.float16
    nc = bacc.Bacc()
    x_d = nc.declare_dram_parameter("x", [L - 1, BPC, N, N], f16, isOutput=False)
    f_d = nc.declare_dram_parameter("features", [BPC, N, D], f16, isOutput=False)
    v0_d = nc.declare_dram_parameter("v0", [N, BPC, 4], f16, isOutput=False)
    cm_d = nc.declare_dram_parameter("cmask", [N, 4], f32, isOutput=False)
    fm_d = nc.declare_dram_parameter("fmask", [N, 4], f32, isOutput=False)
    out_d = nc.declare_dram_parameter("out", [BPC, 4, D], f32, isOutput=True)

    LM = L - 1  # 11 chain layers

    with ExitStack() as ctx:
        tc = ctx.enter_context(tile.TileContext(nc))
        const = ctx.enter_context(tc.tile_pool(name="const", bufs=1))
        xpool = ctx.enter_context(tc.tile_pool(name="xpool", bufs=3))
        fpool = ctx.enter_context(tc.tile_pool(name="fpool", bufs=3))
        vpool = ctx.enter_context(tc.tile_pool(name="vpool", bufs=3))
        opool = ctx.enter_context(tc.tile_pool(name="opool", bufs=8))
        ppool = ctx.enter_context(tc.tile_pool(name="ppool", bufs=2, space="PSUM"))
        cpool = ctx.enter_context(tc.tile_pool(name="cpool", bufs=1, space="PSUM"))

        cmE = const.tile([KE, 4], f32)
        nc.sync.dma_start(out=cmE, in_=cm_d[0:KE])
        cmO = const.tile([KO, 4], f32)
        nc.sync.dma_start(out=cmO, in_=cm_d[KE:N])
        fmE = const.tile([KE, 4], f32)
        nc.sync.dma_start(out=fmE, in_=fm_d[0:KE])
        fmO = const.tile([KO, 4], f32)
        nc.sync.dma_start(out=fmO, in_=fm_d[KE:N])
        # all batches' chain seeds in one load, off the per-batch critical path
        V0E = const.tile([KE, BPC, 4], f16)
        nc.scalar.dma_start(out=V0E, in_=v0_d[0:KE])
        V0O = const.tile([KO, BPC, 4], f16)
        nc.scalar.dma_start(out=V0O, in_=v0_d[KE:N])

        for b in range(BPC):
            # X chain layers, rows paired: partition p holds rows (2p, 2p+1)
            # as 402 contiguous fp16 elements per layer.
            XP = xpool.tile([KE, LM, 2 * N], f16, tag="XP")
            big_eng = nc.sync if b % 2 == 0 else nc.scalar
            small_eng = nc.scalar if b % 2 == 0 else nc.sync
            big_eng.dma_start(
                out=XP[0:KO, :, :],
                in_=x_d[:, b, 0 : N - 1, :].rearrange(
                    "l (p two) j -> p l (two j)", two=2
                ),
            )
            nc.gpsimd.dma_start(
                out=XP[KO:KE, :, 0:N],
                in_=x_d[:, b, N - 1 : N, :].transpose([1, 0, 2]),
            )
            # features, rows paired the same way
            FP = fpool.tile([KE, 2 * D], f16, tag="FP")
            small_eng.dma_start(
                out=FP[0:KO, :],
                in_=f_d[b, 0 : N - 1, :].rearrange("(p two) d -> p (two d)", two=2),
            )
            nc.gpsimd.dma_start(out=FP[KO:KE, 0:D], in_=f_d[b, N - 1 : N, :])
            FR = fpool.tile([KE, 2 * D], f16, tag="FR")
            nc.vector.tensor_scalar_max(FR, FP, 0.0)

            # chain seed: host-precomputed masked+scaled rows 1..4 of layer 11
            vE = V0E[:, b, :]
            vO = V0O[:, b, :]

            for li in range(LM - 1, -1, -1):  # layers 10 .. 0
                pE = ppool.tile([KE, 4], f32, tag="pE")
                pO = ppool.tile([KO, 4], f32, tag="pO")
                # even output cols <- even-row X block + odd-row X block
                nc.tensor.matmul(pE, lhsT=XP[:, li, 0:N:2], rhs=vE, start=True, stop=False)
                nc.tensor.matmul(pE, lhsT=XP[0:KO, li, N : 2 * N : 2], rhs=vO, start=False, stop=True)
                # odd output cols
                nc.tensor.matmul(pO, lhsT=XP[:, li, 1:N:2], rhs=vE, start=True, stop=False)
                nc.tensor.matmul(pO, lhsT=XP[0:KO, li, N + 1 : 2 * N : 2], rhs=vO, start=False, stop=True)
                mE, mO = (cmE, cmO) if li > 0 else (fmE, fmO)
                vE = vpool.tile([KE, 4], f16, tag="vE")
                vO = vpool.tile([KO, 4], f16, tag="vO")
                nc.vector.tensor_mul(vE, pE, mE)
                nc.vector.tensor_mul(vO, pO, mO)

            # CAM: out[4, 768] = V^T @ relu(F); rescale by 16^12 on eviction
            pc0 = cpool.tile([4, H], f32, tag="pc0")
            pc1 = cpool.tile([4, H], f32, tag="pc1")
            nc.tensor.matmul(pc0, lhsT=vE, rhs=FR[:, 0:H], start=True, stop=False)
            nc.tensor.matmul(pc0, lhsT=vO, rhs=FR[0:KO, D : D + H], start=False, stop=True)
            nc.tensor.matmul(pc1, lhsT=vE, rhs=FR[:, H:D], start=True, stop=False)
            nc.tensor.matmul(pc1, lhsT=vO, rhs=FR[0:KO, D + H : 2 * D], start=False, stop=True)
            ob = opool.tile([4, D], f32, tag="ob")
            nc.vector.tensor_scalar_mul(ob[:, 0:H], pc0, SCALE_BACK)
            nc.vector.tensor_scalar_mul(ob[:, H:D], pc1, SCALE_BACK)
            nc.gpsimd.dma_start(out=out_d[b], in_=ob)

    nc.compile()
    return nc


def _get_program():
    if "nc" not in _cache:
        _cache["nc"] = _build_program()
    return _cache["nc"]


def kernel(**inputs):
    from concourse.bass_utils import run_bass_kernel_spmd

    x = np.asarray(inputs["x"])
    features = np.asarray(inputs["features"]).astype(np.float16)
    cmask, fmask = _masks()
    x16 = x[: L - 1].astype(np.float16)
    # host-side chain seed: rows 1..4 of layer 11 -> [b, N, 4], masked,
    # pre-scaled by lambda (a power of two, so order matches on-chip exactly),
    # and permuted to the even/odd on-chip row order.
    v0 = (x[L - 1, :, 1:5, :].transpose(0, 2, 1) * cmask[None]).astype(np.float16)
    # pre-permute rows even/odd and pre-transpose to [N, B, 4] so the on-chip
    # seed DMA is a contiguous partition-major copy
    v0 = np.ascontiguousarray(v0[:, PERM].transpose(1, 0, 2))
    cmaskp = np.ascontiguousarray(cmask[PERM])
    fmaskp = np.ascontiguousarray(fmask[PERM])
    nc = _get_program()
    in_maps = []
    for c in range(NCORES):
        sl = slice(c * BPC, (c + 1) * BPC)
        in_maps.append(
            {
                "x": np.ascontiguousarray(x16[:, sl]),
                "features": np.ascontiguousarray(features[sl]),
                "v0": np.ascontiguousarray(v0[:, sl]),
                "cmask": cmaskp,
                "fmask": fmaskp,
            }
        )
    res = run_bass_kernel_spmd(nc, in_maps, list(range(NCORES)), **RUN_KWARGS)
    _cache["last_result"] = res
    out = np.concatenate([res.results[c]["out"] for c in range(NCORES)], axis=0)
    return out.astype(np.float32)
